# revision 16
# baseline (speedup 1.0000x reference)
"""DCellNN fused Trainium2 kernel (8-core term-sharded / expert-parallel).

Strategy:
  - Fold each term's Linear(KG,H) through its direct-gene Linear(G,KG) on the
    host, so the device runs ONE big GEMM per core: [B,G] x [G, 480-col fused
    block] in float32r (TF32-class, full PE rate), term-sharded 8 ways
    (64 leaves + 8 parents per core; root's gene part on core 0).
  - Terms are sharded contiguously, so parent->children links and training-mode
    BatchNorm batch statistics stay core-local (full batch per term).
  - BN affine (training stats) is folded at runtime into the aux-head /
    next-layer weights, so normalized activations are never materialized.
  - Only cross-core dependency: the root layer's input sum -> one AllReduce of
    a [6, 4096] partial (98KB).
Outputs per core: [74, 4096] = 64 aux0 rows, 8 aux1 rows, aux2, final.
"""
import sys
import os

sys.path.insert(0, '/opt/trn_rl_repo')

import numpy as np

B = 4096
G = 3008
GP = 3072             # G padded to 24 full 128-chunks
T0 = 512
T1 = 64
C = 8
KG = 16
H = 6
TG = T0 + T1 + 1
EPS = 1e-5
N_CORES = 8
LPC = T0 // N_CORES   # 64 leaves per core
PPC = T1 // N_CORES   # 8 parents per core
NB = 8                # batch chunks
BCW = B // NB         # 512
NG = GP // 128        # 24 g-chunks
GG = 4                # g-chunks per DMA group
NGG = NG // GG        # 6 groups
# main-GEMM M-tiles (psum partition rows) and wf column ranges
JR = [126, 126, 126, 102]
JC = [(0, 126), (126, 252), (252, 378), (378, 480)]
WFW = 480
NOUT = 98

# packed small-weight column layout: [128, NSM]
SM_AH = 0             # 4 x 112: per tile j, cols 0:64 aux0 blockdiag, 64:112 w1c
SM_P3 = 448           # 38: cols 0:8 aw1 blockdiag, 32:38 w2loc
SM_AW2 = 486          # 2
SM_VEC = 488          # 35 per-partition vectors
NSM = 523

_CACHE = {}
LAST_RESULTS = None


def _host_prep(gene_input, params):
    f32 = np.float32
    p = {k: np.asarray(v, dtype=f32) for k, v in params.items()}
    x = np.asarray(gene_input, dtype=f32)

    # xT padded to [GP, B], then pre-tiled: xtg[bc, gg, p, k, c] =
    # xT[128*(4*gg+k)+p, 512*bc+c]  -> per-partition-contiguous 8KB DMA groups
    xT = np.zeros((GP, B), f32)
    xT[:G] = x.T
    A = xT.reshape(NGG, GG, 128, NB, BCW)          # [gg, k, p, bc, c]
    xtg = np.ascontiguousarray(A.transpose(3, 0, 2, 1, 4))  # [bc, gg, p, k, c]

    Wg, bg = p['Wg'], p['bg']
    W0, b0 = p['W0'], p['b0']
    W1, b1 = p['W1'], p['b1']
    W2, b2 = p['W2'], p['b2']

    per_core = []
    for i in range(N_CORES):
        ls = slice(LPC * i, LPC * (i + 1))            # global leaves
        ps = slice(T0 + PPC * i, T0 + PPC * (i + 1))  # parent rows in Wg space
        pp = slice(PPC * i, PPC * (i + 1))            # parent index in T1 space

        # --- folded gene weights ---
        WfL = np.matmul(W0[ls], Wg[ls])             # [64, 6, G]
        bfL = np.matmul(W0[ls], bg[ls][..., None])[..., 0] + b0[ls]   # [64, 6]
        W1g = W1[pp][:, :, C * H:]                  # [8, 6, 16]
        WfP = np.matmul(W1g, Wg[ps])                # [8, 6, G]
        bfP = np.matmul(W1g, bg[ps][..., None])[..., 0] + b1[pp]      # [8, 6]
        if i == 0:
            W2g = W2[:, T1 * H:]                    # [6, 16]
            WfR = W2g @ Wg[-1]                      # [6, G]
            bfR = W2g @ bg[-1] + b2                 # [6]
        else:
            WfR = np.zeros((H, G), f32)
            bfR = np.zeros((H,), f32)

        wfT = np.zeros((WFW, GP), f32)
        wfT[0:126, :G] = WfL[0:21].reshape(126, G)
        wfT[126:252, :G] = WfL[21:42].reshape(126, G)
        wfT[252:378, :G] = WfL[42:63].reshape(126, G)
        wfT[378:384, :G] = WfL[63]                  # leaf 63 at tile3 rows 0:6
        wfT[410:458, :G] = WfP.reshape(48, G)       # parents at tile3 rows 32:80
        wfT[474:480, :G] = WfR                      # root at tile3 rows 96:102
        # pre-tiled: wfg[gg, p, k, c] = wfT[c, 128*(4*gg+k)+p]
        Wv = wfT.T.reshape(NGG, GG, 128, WFW)       # [gg, k, p, c]
        wfg = np.ascontiguousarray(Wv.transpose(0, 2, 1, 3))  # [gg, p, k, c]

        # --- packed small weights [128, NSM] ---
        sm = np.zeros((128, NSM), f32)

        aW0l = p['aW0'][ls][:, 0, :]                # [64, 6]
        for j in range(3):
            for lt in range(21):
                sm[6 * lt:6 * lt + 6, SM_AH + 112 * j + 21 * j + lt] = aW0l[21 * j + lt]
        sm[0:6, SM_AH + 112 * 3 + 63] = aW0l[63]

        for gr in range(384):                       # e0 row (local leaf*6 + h')
            l, hp = divmod(gr, 6)
            pl, cc = divmod(l, 8)
            j, r = divmod(gr, 126)
            if j == 3:
                r = gr - 378
            sm[r, SM_AH + 112 * j + 64 + 6 * pl: SM_AH + 112 * j + 64 + 6 * pl + 6] = \
                W1[PPC * i + pl, :, 6 * cc + hp]

        aw1l = p['aW1'][pp][:, 0, :]                # [8, 6]
        for pl in range(8):
            sm[6 * pl:6 * pl + 6, SM_P3 + pl] = aw1l[pl]

        for pl in range(8):
            for h in range(6):
                sm[6 * pl + h, SM_P3 + 32:SM_P3 + 38] = W2[:, (PPC * i + pl) * 6 + h]

        sm[0:6, SM_AW2] = p['aW2'][0]
        sm[0:6, SM_AW2 + 1] = p['Wf'][0]

        # --- per-partition vectors at SM_VEC ---
        g0l, be0l = p['g0'][ls], p['be0'][ls]       # [64, 6]
        ab0l = p['ab0'][ls][:, 0]
        cW0l = p['cW0'][ls][:, 0, 0]
        cb0l = p['cb0'][ls][:, 0]
        vecs = np.zeros((35, 128), f32)
        for j in range(3):
            sl = slice(21 * j, 21 * (j + 1))
            vecs[0 + j, 0:126] = bfL[sl].reshape(126)
            vecs[4 + j, 0:126] = g0l[sl].reshape(126)
            vecs[8 + j, 0:126] = be0l[sl].reshape(126)

        # tile3: bias laid out like the psum rows (0:6 leaf, 32:80 parents, 96:102 root)
        vecs[3, 0:6] = bfL[63]
        vecs[3, 32:80] = bfP.reshape(48)
        vecs[3, 96:102] = bfR
        vecs[7, 0:6] = g0l[63]
        vecs[11, 0:6] = be0l[63]
        vecs[12, 0:64] = ab0l
        vecs[16, 0:64] = cW0l
        vecs[20, 0:64] = cb0l
        vecs[24, 0:48] = p['g1'][pp].reshape(48)
        vecs[25, 0:48] = p['be1'][pp].reshape(48)
        vecs[26, 0:8] = p['ab1'][pp][:, 0]
        vecs[27, 0:8] = p['cW1'][pp][:, 0, 0]
        vecs[28, 0:8] = p['cb1'][pp][:, 0]
        vecs[29, 0:6] = p['g2']
        vecs[30, 0:6] = p['be2']
        vecs[31, 0:2] = [p['ab2'][0], p['bf'][0]]
        vecs[32, 0:2] = [p['cW2'][0, 0], p['Wo'][0, 0]]
        vecs[33, 0:2] = [p['cb2'][0], p['bo'][0]]
        vecs[34, :] = EPS
        sm[:, SM_VEC:SM_VEC + 35] = vecs.T

        per_core.append(dict(xt=xtg, wf=wfg, sm=sm))
    return per_core


def _build_program():
    import concourse.bass as bass
    import concourse.bacc as bacc
    import concourse.mybir as mybir
    import concourse.tile as tile
    from concourse.tile import add_dep_helper

    F32 = mybir.dt.float32
    F32R = mybir.dt.float32r
    Tanh = mybir.ActivationFunctionType.Tanh
    Sqrt = mybir.ActivationFunctionType.Sqrt
    Ident = mybir.ActivationFunctionType.Identity
    Alu = mybir.AluOpType

    nc = bacc.Bacc("TRN2", target_bir_lowering=False, debug=False,
                   num_devices=N_CORES)
    xt_d = nc.dram_tensor("xt", [NB, NGG, 128, GG * BCW], F32R,
                          kind="ExternalInput").ap()
    wf_d = nc.dram_tensor("wf", [NGG, 128, GG * WFW], F32R,
                          kind="ExternalInput").ap()
    sm_d = nc.dram_tensor("sm", [128, NSM], F32, kind="ExternalInput").ap()
    out_d = nc.dram_tensor("out", [NOUT, B], F32, kind="ExternalOutput").ap()

    with tile.TileContext(nc) as tc:
        from contextlib import ExitStack
        with ExitStack() as ctx:
            cw = ctx.enter_context(tc.tile_pool(name="cw", bufs=1))
            xp = ctx.enter_context(tc.tile_pool(name="xp", bufs=2))
            big = ctx.enter_context(tc.tile_pool(name="big", bufs=1))
            tmp = ctx.enter_context(tc.tile_pool(name="tmp", bufs=2))
            drp = ctx.enter_context(tc.tile_pool(name="drp", bufs=2, space="DRAM"))

            # ---- constants ----
            wf_sb = cw.tile([128, NG * WFW], F32R, tag="wf")
            for gg in range(NGG):
                nc.gpsimd.dma_start(
                    wf_sb[:, gg * GG * WFW:(gg + 1) * GG * WFW], wf_d[gg])
            sm_sb = cw.tile([128, NSM], F32, tag="sm")
            nc.gpsimd.dma_start(sm_sb[:], sm_d[:])

            # warm up the collectives firmware with a dummy AllReduce: the
            # first collective pays ~65us of mesh init; overlap it with the
            # main GEMM so the real AllReduce later costs ~14us.
            d_in = drp.tile([1, 128], F32, name="din")
            d_out = drp.tile([1, 128], F32, name="dout")
            zz = tmp.tile([1, 128], F32, tag="zz", name="zz")
            nc.gpsimd.memset(zz[:], 1.0)
            nc.gpsimd.dma_start(d_in[:], zz[:])
            nc.gpsimd.collective_compute(
                "AllReduce", Alu.add,
                replica_groups=[list(range(N_CORES))],
                ins=[d_in.opt()], outs=[d_out.opt()])

            def vec(k):
                return sm_sb[:, SM_VEC + k:SM_VEC + k + 1]

            # vector indices
            V_B0, V_G0, V_BE0, V_AB0, V_CW0, V_CB0 = 0, 4, 8, 12, 16, 20
            V_G1, V_BE1, V_AB1, V_CW1, V_CB1 = 24, 25, 26, 27, 28
            V_G2, V_BE2, V_HDB, V_HDS, V_HDSH = 29, 30, 31, 32, 33
            V_EPS = 34

            def ahb(j):
                n = 126 if j < 3 else 6
                return sm_sb[0:n, SM_AH + 112 * j:SM_AH + 112 * (j + 1)]

            def aw0b(j):
                n = 126 if j < 3 else 6
                return sm_sb[0:n, SM_AH + 112 * j:SM_AH + 112 * j + 64]

            def w1cb(j):
                n = 126 if j < 3 else 6
                return sm_sb[0:n, SM_AH + 112 * j + 64:SM_AH + 112 * (j + 1)]

            p3b = sm_sb[0:48, SM_P3:SM_P3 + 38]
            aw1b = sm_sb[0:48, SM_P3:SM_P3 + 8]
            w2locb = sm_sb[0:48, SM_P3 + 32:SM_P3 + 38]
            aw2f = sm_sb[0:6, SM_AW2:SM_AW2 + 2]

            # ---- persistent activations ----
            tanh0 = [big.tile([126, B], F32R, tag=f"th{j}", name=f"th{j}")
                     for j in range(3)]
            t3a = big.tile([6, B], F32R, tag="t3a")       # tanh(leaf 63)
            rawh1 = big.tile([48, B], F32, tag="rawh1")   # parents' x-part + bias
            rawroot = big.tile([6, B], F32, tag="rawroot")
            tanh1 = big.tile([48, B], F32R, tag="tanh1")
            out_sb = big.tile([NOUT, B], F32, tag="outsb")
            stats0 = [cw.tile([126, 6 * NB], F32, tag=f"s0{j}", name=f"s0{j}")
                      for j in range(3)]
            stats3 = cw.tile([6, 6 * NB], F32, tag="s03")
            stats1 = cw.tile([48, 6 * NB], F32, tag="s1")
            stats2 = cw.tile([6, 6 * NB], F32, tag="s2")
            h2p = big.tile([6, B], F32, tag="th2", name="h2p")

            # ================= phase 1: main fused GEMM =================
            with tc.tile_pool(name="mps", bufs=8, space="PSUM") as mps:
                for bc in range(NB):
                    bs = slice(BCW * bc, BCW * (bc + 1))
                    ps = [mps.tile([JR[j], BCW], F32, tag="mps", name=f"ps{bc}_{j}")
                          for j in range(4)]
                    for gg in range(NGG):
                        xt_t = xp.tile([128, GG * BCW], F32R, tag="xt")
                        nc.sync.dma_start(xt_t[:], xt_d[bc, gg])
                        for j in range(4):
                            c0, c1 = JC[j]
                            for k in range(GG):
                                g = GG * gg + k
                                nc.tensor.matmul(
                                    ps[j][:, :],
                                    wf_sb[:, g * WFW + c0:g * WFW + c1],
                                    xt_t[:, k * BCW:(k + 1) * BCW],
                                    start=(g == 0), stop=(g == NG - 1))
                    sl6 = slice(6 * bc, 6 * (bc + 1))
                    for j in range(3):
                        nc.scalar.activation(tanh0[j][:, bs], ps[j][:, :], Tanh,
                                             bias=vec(V_B0 + j)[0:126])
                        nc.vector.bn_stats(stats0[j][:, sl6], tanh0[j][:, bs])
                    nc.scalar.activation(t3a[:, bs], ps[3][0:6, :], Tanh,
                                         bias=vec(V_B0 + 3)[0:6])
                    nc.vector.bn_stats(stats3[:, sl6], t3a[:, bs])
                    nc.scalar.activation(rawh1[0:32, bs], ps[3][32:64, :], Ident,
                                         bias=vec(V_B0 + 3)[32:64])
                    nc.scalar.activation(rawh1[32:48, bs], ps[3][64:80, :], Ident,
                                         bias=vec(V_B0 + 3)[64:80])
                    nc.scalar.activation(rawroot[:, bs], ps[3][96:102, :], Ident,
                                         bias=vec(V_B0 + 3)[96:102])

            sps = ctx.enter_context(tc.tile_pool(name="sps", bufs=2, space="PSUM"))

            # ================= BN folds =================
            def bn_fold(stats, n, vg, vbe, tagp):
                mv = tmp.tile([n, 2], F32, tag="mv", name=f"mv_{tagp}")
                nc.vector.bn_aggr(mv[:], stats[:n, :])
                sq = tmp.tile([n, 1], F32, tag="sq", name=f"sq_{tagp}")
                nc.scalar.activation(sq[:], mv[:, 1:2], Sqrt, bias=vec(V_EPS)[0:n])
                rs = tmp.tile([n, 1], F32, tag="rs", name=f"rs_{tagp}")
                nc.vector.reciprocal(rs[:], sq[:])
                s = cw.tile([n, 1], F32, tag=f"s_{tagp}", name=f"s_{tagp}")
                nc.vector.tensor_mul(s[:], rs[:], vg[0:n])
                t = tmp.tile([n, 1], F32, tag="tm", name=f"tm_{tagp}")
                nc.vector.tensor_mul(t[:], mv[:, 0:1], s[:])
                c = cw.tile([n, 1], F32, tag=f"c_{tagp}", name=f"c_{tagp}")
                nc.vector.tensor_sub(c[:], vbe[0:n], t[:])
                return s, c

            s0 = [None] * 4
            c0 = [None] * 4
            for j in range(3):
                s0[j], c0[j] = bn_fold(stats0[j], 126, vec(V_G0 + j),
                                       vec(V_BE0 + j), f"bn0{j}")
            s0[3], c0[3] = bn_fold(stats3, 6, vec(V_G0 + 3), vec(V_BE0 + 3), "bn03")

            JN = [126, 126, 126, 6]     # contraction rows per tile for heads
            aw0f = []
            w1cf = []
            for j in range(4):
                n = JN[j]
                f = cw.tile([n, 64], F32R, tag=f"aw0f{j}", name=f"aw0f{j}")
                nc.vector.tensor_scalar(f[:], aw0b(j), s0[j][:], None, Alu.mult)
                aw0f.append(f)
                f = cw.tile([n, 48], F32R, tag=f"w1cf{j}", name=f"w1cf{j}")
                nc.vector.tensor_scalar(f[:], w1cb(j), s0[j][:], None, Alu.mult)
                w1cf.append(f)
            a0cps = sps.tile([64, 1], F32, tag="sp1", bufs=2)
            for j in range(4):
                nc.tensor.matmul(a0cps[:, :], aw0b(j), c0[j][:, :],
                                 start=(j == 0), stop=(j == 3))
            ab0p = cw.tile([64, 1], F32, tag="ab0p")
            nc.vector.tensor_add(ab0p[:], a0cps[:, :], vec(V_AB0)[0:64])
            b1ps = sps.tile([48, 1], F32, tag="sp1", bufs=2)
            for j in range(4):
                nc.tensor.matmul(b1ps[:, :], w1cb(j), c0[j][:, :],
                                 start=(j == 0), stop=(j == 3))
            b1c = cw.tile([48, 1], F32, tag="b1c")
            nc.vector.tensor_copy(b1c[:], b1ps[:, :])

            # ================= phase 2a: h1 (child contributions) ==============
            rhs0 = [tanh0[0], tanh0[1], tanh0[2], t3a]
            for bc in range(NB):
                bs = slice(BCW * bc, BCW * (bc + 1))
                h1ps = sps.tile([48, BCW], F32, tag="sp2", name=f"h1ps{bc}",
                                bufs=4)
                for j in range(4):
                    nc.tensor.matmul(h1ps[:, :], w1cf[j][:, :], rhs0[j][:JN[j], bs],
                                     start=(j == 0), stop=(j == 3))
                tm1 = tmp.tile([48, BCW], F32, tag="work", name=f"tm1{bc}", bufs=4)
                nc.vector.scalar_tensor_tensor(tm1[:, :], h1ps[:, :],
                                               b1c[:], rawh1[:, bs],
                                               Alu.add, Alu.add)
                nc.scalar.activation(tanh1[:, bs], tm1[:], Tanh)
                nc.vector.bn_stats(stats1[:, slice(6 * bc, 6 * bc + 6)],
                                   tanh1[:, bs])

            # ================= BN1 folds =================
            s1, c1 = bn_fold(stats1, 48, vec(V_G1), vec(V_BE1), "bn1")
            w2locf = cw.tile([48, 6], F32R, tag="w2locf")
            nc.vector.tensor_scalar(w2locf[:], w2locb, s1[:], None, Alu.mult)
            aw1f = cw.tile([48, 8], F32R, tag="aw1f")
            nc.vector.tensor_scalar(aw1f[:], aw1b, s1[:], None, Alu.mult)
            cps1 = sps.tile([8, 1], F32, tag="sp1", bufs=2)
            nc.tensor.matmul(cps1[:, :], aw1b, c1[:, :], start=True, stop=True)
            ab1p = cw.tile([8, 1], F32, tag="ab1p")
            nc.vector.tensor_add(ab1p[:], cps1[:, :], vec(V_AB1)[0:8])
            rps = sps.tile([6, 1], F32, tag="sp1", bufs=2)
            nc.tensor.matmul(rps[:, :], w2locb, c1[:, :], start=True, stop=True)
            rootc = cw.tile([6, 1], F32, tag="rootc")
            nc.vector.tensor_copy(rootc[:], rps[:, :])

            # ================= phase 3a: root partial + per-chunk bounce =======
            arin = drp.tile([6, B], F32)
            arout = drp.tile([6, B], F32)
            for bc in range(NB):
                bs = slice(BCW * bc, BCW * (bc + 1))
                rp = sps.tile([6, BCW], F32, tag="sp2", name=f"rps{bc}", bufs=4)
                rp_mm = nc.tensor.matmul(rp[:, :], w2locf[:, :], tanh1[:, bs],
                                         start=True, stop=True)
                nc.vector.scalar_tensor_tensor(h2p[:, bs], rp[:, :], rootc[:],
                                               rawroot[:, bs], Alu.add, Alu.add)

            # ================= phase 4: AllReduce of root partial =================
            nc.gpsimd.dma_start(arin[:], h2p[:])
            nc.gpsimd.collective_compute(
                "AllReduce", Alu.add,
                replica_groups=[list(range(N_CORES))],
                ins=[arin.opt()], outs=[arout.opt()])

            # ======== aux0 + aux1 heads, in flight during the AllReduce ========
            for bc in range(NB):
                bs = slice(BCW * bc, BCW * (bc + 1))
                a0ps = sps.tile([64, BCW], F32, tag="sp3", name=f"a0ps{bc}",
                                bufs=2)
                for j in range(4):
                    mmh = nc.tensor.matmul(a0ps[:, :], aw0f[j][:, :],
                                           rhs0[j][:JN[j], bs],
                                           start=(j == 0), stop=(j == 3))
                    if j == 0:
                        add_dep_helper(rp_mm.ins, mmh.ins, sync=False,
                                       reason="aux fills AllReduce flight")
                a0sb = tmp.tile([64, BCW], F32, tag="work", name=f"a0sb{bc}", bufs=4)
                nc.scalar.activation(a0sb[:], a0ps[:, :], Tanh, bias=ab0p[:])
                nc.vector.tensor_scalar(out_sb[0:64, bs], a0sb[:],
                                        vec(V_CW0)[0:64],
                                        vec(V_CB0)[0:64], Alu.mult, Alu.add)
                a1ps = sps.tile([8, BCW], F32, tag="sp3", name=f"a1ps{bc}", bufs=2)
                mmh1 = nc.tensor.matmul(a1ps[:, :], aw1f[:, :], tanh1[:, bs],
                                        start=True, stop=True)
                add_dep_helper(rp_mm.ins, mmh1.ins, sync=False,
                               reason="aux fills AllReduce flight")
                a1sb = tmp.tile([8, BCW], F32, tag="work", name=f"a1sb{bc}", bufs=4)
                nc.scalar.activation(a1sb[:], a1ps[:, :], Tanh, bias=ab1p[:])
                nc.vector.tensor_scalar(out_sb[64:72, bs], a1sb[:],
                                        vec(V_CW1)[0:8], vec(V_CB1)[0:8],
                                        Alu.mult, Alu.add)
            nc.gpsimd.dma_start(out_d[0:72, :], out_sb[0:72, :])
            h2s = big.tile([6, B], F32, tag="th0", name="h2s")
            nc.scalar.dma_start(h2s[:], arout[:])

            # ================= phase 5: root head =================
            tanh2 = big.tile([6, B], F32R, tag="th1", name="tanh2")
            nc.scalar.activation(tanh2[:], h2s[:], Tanh)
            for bc in range(NB):
                bs = slice(BCW * bc, BCW * (bc + 1))
                nc.vector.bn_stats(stats2[:, slice(6 * bc, 6 * bc + 6)],
                                   tanh2[:, bs])
            s2, c2 = bn_fold(stats2, 6, vec(V_G2), vec(V_BE2), "bn2")
            aw2ff = cw.tile([6, 2], F32R, tag="aw2ff")
            nc.vector.tensor_scalar(aw2ff[:], aw2f, s2[:], None, Alu.mult)
            hps = sps.tile([2, 1], F32, tag="sp1", bufs=2)
            nc.tensor.matmul(hps[:, :], aw2f, c2[:, :], start=True, stop=True)
            hdbp = cw.tile([2, 1], F32, tag="hdbp")
            nc.vector.tensor_add(hdbp[:], hps[:, :], vec(V_HDB)[0:2])
            for bc in range(NB):
                bs = slice(BCW * bc, BCW * (bc + 1))
                hp2 = sps.tile([2, BCW], F32, tag="sp2", name=f"hps{bc}", bufs=4)
                nc.tensor.matmul(hp2[:, :], aw2ff[:, :], tanh2[:, bs],
                                 start=True, stop=True)
                hsb = tmp.tile([2, BCW], F32, tag="work", name=f"hsb{bc}", bufs=4)
                nc.scalar.activation(hsb[:], hp2[:, :], Tanh, bias=hdbp[:])
                nc.vector.tensor_scalar(out_sb[96:98, bs], hsb[:],
                                        vec(V_HDS)[0:2], vec(V_HDSH)[0:2],
                                        Alu.mult, Alu.add)

            # ================= output (head rows) =================
            nc.gpsimd.dma_start(out_d[96:98, :], out_sb[96:98, :])

    nc.compile()
    return nc


def kernel(gene_input, params):
    global LAST_RESULTS
    from concourse import bass_utils

    if 'nc' not in _CACHE:
        _CACHE['nc'] = _build_program()
    nc = _CACHE['nc']

    per_core = _host_prep(gene_input, params)
    in_maps = [dict(xt=pc['xt'], wf=pc['wf'], sm=pc['sm']) for pc in per_core]

    res = bass_utils.run_bass_kernel_spmd(
        nc, in_maps, core_ids=list(range(N_CORES)),
        trace=bool(os.environ.get('KERNEL_TRACE')))
    LAST_RESULTS = res

    full = np.empty((B, T0 + T1 + 2), dtype=np.float32)
    for c in range(N_CORES):
        o = res.results[c]['out']
        full[:, LPC * c:LPC * (c + 1)] = o[0:64].T
        full[:, T0 + PPC * c:T0 + PPC * (c + 1)] = o[64:72].T
    o0 = res.results[0]['out']
    full[:, T0 + T1] = o0[96]
    full[:, T0 + T1 + 1] = o0[97]
    return full


# revision 18
# speedup vs baseline: 1.2082x; 1.2082x over previous
"""DCellNN fused Trainium2 kernel (8-core term-sharded / expert-parallel).

Strategy:
  - Fold each term's Linear(KG,H) through its direct-gene Linear(G,KG) on the
    host, so the device runs ONE big GEMM per core: [B,G] x [G, 480-col fused
    block] in float32r (TF32-class, full PE rate), term-sharded 8 ways
    (64 leaves + 8 parents per core; root's gene part on core 0).
  - Terms are sharded contiguously, so parent->children links and training-mode
    BatchNorm batch statistics stay core-local (full batch per term).
  - BN affine (training stats) is folded at runtime into the aux-head /
    next-layer weights, so normalized activations are never materialized.
  - Only cross-core dependency: the root layer's input sum -> one AllReduce of
    a [6, 4096] partial (98KB).
Outputs per core: [74, 4096] = 64 aux0 rows, 8 aux1 rows, aux2, final.
"""
import sys
import os

sys.path.insert(0, '/opt/trn_rl_repo')

import numpy as np

B = 4096
G = 3008
GP = 3072             # G padded to 24 full 128-chunks
T0 = 512
T1 = 64
C = 8
KG = 16
H = 6
TG = T0 + T1 + 1
EPS = 1e-5
N_CORES = 8
LPC = T0 // N_CORES   # 64 leaves per core
PPC = T1 // N_CORES   # 8 parents per core
NB = 8                # batch chunks
BCW = B // NB         # 512
NG = GP // 128        # 24 g-chunks
GG = 4                # g-chunks per DMA group
NGG = NG // GG        # 6 groups
# main-GEMM M-tiles (psum partition rows) and wf column ranges
JR = [126, 126, 126, 102]
JC = [(0, 126), (126, 252), (252, 378), (378, 480)]
WFW = 480
NOUT = 98

# packed small-weight column layout: [128, NSM]
SM_AH = 0             # 4 x 112: per tile j, cols 0:64 aux0 blockdiag, 64:112 w1c
SM_P3 = 448           # 38: cols 0:8 aw1 blockdiag, 32:38 w2loc
SM_AW2 = 486          # 2
SM_VEC = 488          # 35 per-partition vectors
NSM = 523

_CACHE = {}
LAST_RESULTS = None


def _host_prep(gene_input, params):
    f32 = np.float32
    p = {k: np.asarray(v, dtype=f32) for k, v in params.items()}
    x = np.asarray(gene_input, dtype=f32)

    # xT padded to [GP, B], then pre-tiled: xtg[bc, gg, p, k, c] =
    # xT[128*(4*gg+k)+p, 512*bc+c]  -> per-partition-contiguous 8KB DMA groups
    xT = np.zeros((GP, B), f32)
    xT[:G] = x.T
    A = xT.reshape(NGG, GG, 128, NB, BCW)          # [gg, k, p, bc, c]
    xtg = np.ascontiguousarray(A.transpose(3, 0, 2, 1, 4))  # [bc, gg, p, k, c]

    Wg, bg = p['Wg'], p['bg']
    W0, b0 = p['W0'], p['b0']
    W1, b1 = p['W1'], p['b1']
    W2, b2 = p['W2'], p['b2']

    per_core = []
    for i in range(N_CORES):
        ls = slice(LPC * i, LPC * (i + 1))            # global leaves
        ps = slice(T0 + PPC * i, T0 + PPC * (i + 1))  # parent rows in Wg space
        pp = slice(PPC * i, PPC * (i + 1))            # parent index in T1 space

        # --- folded gene weights ---
        WfL = np.matmul(W0[ls], Wg[ls])             # [64, 6, G]
        bfL = np.matmul(W0[ls], bg[ls][..., None])[..., 0] + b0[ls]   # [64, 6]
        W1g = W1[pp][:, :, C * H:]                  # [8, 6, 16]
        WfP = np.matmul(W1g, Wg[ps])                # [8, 6, G]
        bfP = np.matmul(W1g, bg[ps][..., None])[..., 0] + b1[pp]      # [8, 6]
        if i == 0:
            W2g = W2[:, T1 * H:]                    # [6, 16]
            WfR = W2g @ Wg[-1]                      # [6, G]
            bfR = W2g @ bg[-1] + b2                 # [6]
        else:
            WfR = np.zeros((H, G), f32)
            bfR = np.zeros((H,), f32)

        wfT = np.zeros((WFW, GP), f32)
        wfT[0:126, :G] = WfL[0:21].reshape(126, G)
        wfT[126:252, :G] = WfL[21:42].reshape(126, G)
        wfT[252:378, :G] = WfL[42:63].reshape(126, G)
        wfT[378:384, :G] = WfL[63]                  # leaf 63 at tile3 rows 0:6
        wfT[410:458, :G] = WfP.reshape(48, G)       # parents at tile3 rows 32:80
        wfT[474:480, :G] = WfR                      # root at tile3 rows 96:102
        # pre-tiled: wfg[gg, p, k, c] = wfT[c, 128*(4*gg+k)+p]
        Wv = wfT.T.reshape(NGG, GG, 128, WFW)       # [gg, k, p, c]
        wfg = np.ascontiguousarray(Wv.transpose(0, 2, 1, 3))  # [gg, p, k, c]

        # --- packed small weights [128, NSM] ---
        sm = np.zeros((128, NSM), f32)

        aW0l = p['aW0'][ls][:, 0, :]                # [64, 6]
        for j in range(3):
            for lt in range(21):
                sm[6 * lt:6 * lt + 6, SM_AH + 112 * j + 21 * j + lt] = aW0l[21 * j + lt]
        sm[0:6, SM_AH + 112 * 3 + 63] = aW0l[63]

        for gr in range(384):                       # e0 row (local leaf*6 + h')
            l, hp = divmod(gr, 6)
            pl, cc = divmod(l, 8)
            j, r = divmod(gr, 126)
            if j == 3:
                r = gr - 378
            sm[r, SM_AH + 112 * j + 64 + 6 * pl: SM_AH + 112 * j + 64 + 6 * pl + 6] = \
                W1[PPC * i + pl, :, 6 * cc + hp]

        aw1l = p['aW1'][pp][:, 0, :]                # [8, 6]
        for pl in range(8):
            sm[6 * pl:6 * pl + 6, SM_P3 + pl] = aw1l[pl]

        for pl in range(8):
            for h in range(6):
                sm[6 * pl + h, SM_P3 + 32:SM_P3 + 38] = W2[:, (PPC * i + pl) * 6 + h]

        sm[0:6, SM_AW2] = p['aW2'][0]
        sm[0:6, SM_AW2 + 1] = p['Wf'][0]

        # --- per-partition vectors at SM_VEC ---
        g0l, be0l = p['g0'][ls], p['be0'][ls]       # [64, 6]
        ab0l = p['ab0'][ls][:, 0]
        cW0l = p['cW0'][ls][:, 0, 0]
        cb0l = p['cb0'][ls][:, 0]
        vecs = np.zeros((35, 128), f32)
        for j in range(3):
            sl = slice(21 * j, 21 * (j + 1))
            vecs[0 + j, 0:126] = bfL[sl].reshape(126)
            vecs[4 + j, 0:126] = g0l[sl].reshape(126)
            vecs[8 + j, 0:126] = be0l[sl].reshape(126)

        # tile3: bias laid out like the psum rows (0:6 leaf, 32:80 parents, 96:102 root)
        vecs[3, 0:6] = bfL[63]
        vecs[3, 32:80] = bfP.reshape(48)
        vecs[3, 96:102] = bfR
        vecs[7, 0:6] = g0l[63]
        vecs[11, 0:6] = be0l[63]
        vecs[12, 0:64] = ab0l
        vecs[16, 0:64] = cW0l
        vecs[20, 0:64] = cb0l
        vecs[24, 0:48] = p['g1'][pp].reshape(48)
        vecs[25, 0:48] = p['be1'][pp].reshape(48)
        vecs[26, 0:8] = p['ab1'][pp][:, 0]
        vecs[27, 0:8] = p['cW1'][pp][:, 0, 0]
        vecs[28, 0:8] = p['cb1'][pp][:, 0]
        vecs[29, 0:6] = p['g2']
        vecs[30, 0:6] = p['be2']
        vecs[31, 0:2] = [p['ab2'][0], p['bf'][0]]
        vecs[32, 0:2] = [p['cW2'][0, 0], p['Wo'][0, 0]]
        vecs[33, 0:2] = [p['cb2'][0], p['bo'][0]]
        vecs[34, :] = EPS
        sm[:, SM_VEC:SM_VEC + 35] = vecs.T

        per_core.append(dict(xt=xtg, wf=wfg, sm=sm))
    return per_core


def _build_program():
    import concourse.bass as bass
    import concourse.bacc as bacc
    import concourse.mybir as mybir
    import concourse.tile as tile
    from concourse.tile import add_dep_helper

    F32 = mybir.dt.float32
    F32R = mybir.dt.float32r
    Tanh = mybir.ActivationFunctionType.Tanh
    Sqrt = mybir.ActivationFunctionType.Sqrt
    Ident = mybir.ActivationFunctionType.Identity
    Alu = mybir.AluOpType

    nc = bacc.Bacc("TRN2", target_bir_lowering=False, debug=False,
                   num_devices=N_CORES)
    xt_d = nc.dram_tensor("xt", [NB, NGG, 128, GG * BCW], F32R,
                          kind="ExternalInput").ap()
    wf_d = nc.dram_tensor("wf", [NGG, 128, GG * WFW], F32R,
                          kind="ExternalInput").ap()
    sm_d = nc.dram_tensor("sm", [128, NSM], F32, kind="ExternalInput").ap()
    out_d = nc.dram_tensor("out", [NOUT, B], F32, kind="ExternalOutput").ap()

    with tile.TileContext(nc) as tc:
        from contextlib import ExitStack
        with ExitStack() as ctx:
            cw = ctx.enter_context(tc.tile_pool(name="cw", bufs=1))
            xp = ctx.enter_context(tc.tile_pool(name="xp", bufs=3))
            big = ctx.enter_context(tc.tile_pool(name="big", bufs=1))
            tmp = ctx.enter_context(tc.tile_pool(name="tmp", bufs=2))
            drp = ctx.enter_context(tc.tile_pool(name="drp", bufs=2, space="DRAM"))

            # ---- constants ----
            wf_sb = cw.tile([128, NG * WFW], F32R, tag="wf")
            for gg in range(NGG):
                nc.gpsimd.dma_start(
                    wf_sb[:, gg * GG * WFW:(gg + 1) * GG * WFW], wf_d[gg])
            sm_sb = cw.tile([128, NSM], F32, tag="sm")
            nc.gpsimd.dma_start(sm_sb[:], sm_d[:])

            # warm up the collectives firmware with a dummy AllReduce: the
            # first collective pays ~65us of mesh init; overlap it with the
            # main GEMM so the real AllReduce later costs ~14us.
            d_in = drp.tile([1, 128], F32, name="din")
            d_out = drp.tile([1, 128], F32, name="dout")
            zz = tmp.tile([1, 128], F32, tag="zz", name="zz")
            nc.gpsimd.memset(zz[:], 1.0)
            nc.gpsimd.dma_start(d_in[:], zz[:])
            nc.gpsimd.collective_compute(
                "AllReduce", Alu.add,
                replica_groups=[list(range(N_CORES))],
                ins=[d_in.opt()], outs=[d_out.opt()])

            def vec(k):
                return sm_sb[:, SM_VEC + k:SM_VEC + k + 1]

            # vector indices
            V_B0, V_G0, V_BE0, V_AB0, V_CW0, V_CB0 = 0, 4, 8, 12, 16, 20
            V_G1, V_BE1, V_AB1, V_CW1, V_CB1 = 24, 25, 26, 27, 28
            V_G2, V_BE2, V_HDB, V_HDS, V_HDSH = 29, 30, 31, 32, 33
            V_EPS = 34

            def ahb(j):
                n = 126 if j < 3 else 6
                return sm_sb[0:n, SM_AH + 112 * j:SM_AH + 112 * (j + 1)]

            def aw0b(j):
                n = 126 if j < 3 else 6
                return sm_sb[0:n, SM_AH + 112 * j:SM_AH + 112 * j + 64]

            def w1cb(j):
                n = 126 if j < 3 else 6
                return sm_sb[0:n, SM_AH + 112 * j + 64:SM_AH + 112 * (j + 1)]

            p3b = sm_sb[0:48, SM_P3:SM_P3 + 38]
            aw1b = sm_sb[0:48, SM_P3:SM_P3 + 8]
            w2locb = sm_sb[0:48, SM_P3 + 32:SM_P3 + 38]
            aw2f = sm_sb[0:6, SM_AW2:SM_AW2 + 2]

            # ---- persistent activations ----
            tanh0 = [big.tile([126, B], F32R, tag=f"th{j}", name=f"th{j}")
                     for j in range(3)]
            t3a = big.tile([6, B], F32R, tag="t3a")       # tanh(leaf 63)
            rawh1 = big.tile([48, B], F32, tag="rawh1")   # parents' x-part + bias
            rawroot = big.tile([6, B], F32, tag="rawroot")
            tanh1 = big.tile([48, B], F32R, tag="tanh1")
            out_sb = big.tile([NOUT, B], F32, tag="outsb")
            stats0 = [cw.tile([126, 6 * NB], F32, tag=f"s0{j}", name=f"s0{j}")
                      for j in range(3)]
            stats3 = cw.tile([6, 6 * NB], F32, tag="s03")
            stats1 = cw.tile([48, 6 * NB], F32, tag="s1")
            stats2 = cw.tile([6, 6 * NB], F32, tag="s2")
            h2p = big.tile([6, B], F32, tag="rawh1", name="h2p")

            # ================= phase 1: main fused GEMM =================
            with tc.tile_pool(name="mps", bufs=8, space="PSUM") as mps:
                for bc in range(NB):
                    bs = slice(BCW * bc, BCW * (bc + 1))
                    ps = [mps.tile([JR[j], BCW], F32, tag="mps", name=f"ps{bc}_{j}")
                          for j in range(4)]
                    for gg in range(NGG):
                        xt_t = xp.tile([128, GG * BCW], F32R, tag="xt")
                        nc.sync.dma_start(xt_t[:], xt_d[bc, gg])
                        for j in range(4):
                            c0, c1 = JC[j]
                            for k in range(GG):
                                g = GG * gg + k
                                nc.tensor.matmul(
                                    ps[j][:, :],
                                    wf_sb[:, g * WFW + c0:g * WFW + c1],
                                    xt_t[:, k * BCW:(k + 1) * BCW],
                                    start=(g == 0), stop=(g == NG - 1))
                    sl6 = slice(6 * bc, 6 * (bc + 1))
                    for j in range(3):
                        nc.scalar.activation(tanh0[j][:, bs], ps[j][:, :], Tanh,
                                             bias=vec(V_B0 + j)[0:126])
                        nc.vector.bn_stats(stats0[j][:, sl6], tanh0[j][:, bs])
                    nc.scalar.activation(t3a[:, bs], ps[3][0:6, :], Tanh,
                                         bias=vec(V_B0 + 3)[0:6])
                    nc.vector.bn_stats(stats3[:, sl6], t3a[:, bs])
                    nc.scalar.activation(rawh1[0:32, bs], ps[3][32:64, :], Ident,
                                         bias=vec(V_B0 + 3)[32:64])
                    nc.scalar.activation(rawh1[32:48, bs], ps[3][64:80, :], Ident,
                                         bias=vec(V_B0 + 3)[64:80])
                    nc.scalar.activation(rawroot[:, bs], ps[3][96:102, :], Ident,
                                         bias=vec(V_B0 + 3)[96:102])

            sps = ctx.enter_context(tc.tile_pool(name="sps", bufs=2, space="PSUM"))

            # ================= BN folds =================
            def bn_fold(stats, n, vg, vbe, tagp):
                mv = tmp.tile([n, 2], F32, tag="mv", name=f"mv_{tagp}")
                nc.vector.bn_aggr(mv[:], stats[:n, :])
                sq = tmp.tile([n, 1], F32, tag="sq", name=f"sq_{tagp}")
                nc.scalar.activation(sq[:], mv[:, 1:2], Sqrt, bias=vec(V_EPS)[0:n])
                rs = tmp.tile([n, 1], F32, tag="rs", name=f"rs_{tagp}")
                nc.vector.reciprocal(rs[:], sq[:])
                s = cw.tile([n, 1], F32, tag=f"s_{tagp}", name=f"s_{tagp}")
                nc.vector.tensor_mul(s[:], rs[:], vg[0:n])
                t = tmp.tile([n, 1], F32, tag="tm", name=f"tm_{tagp}")
                nc.vector.tensor_mul(t[:], mv[:, 0:1], s[:])
                c = cw.tile([n, 1], F32, tag=f"c_{tagp}", name=f"c_{tagp}")
                nc.vector.tensor_sub(c[:], vbe[0:n], t[:])
                return s, c

            s0 = [None] * 4
            c0 = [None] * 4
            for j in range(3):
                s0[j], c0[j] = bn_fold(stats0[j], 126, vec(V_G0 + j),
                                       vec(V_BE0 + j), f"bn0{j}")
            s0[3], c0[3] = bn_fold(stats3, 6, vec(V_G0 + 3), vec(V_BE0 + 3), "bn03")

            JN = [126, 126, 126, 6]     # contraction rows per tile for heads
            aw0f = []
            w1cf = []
            for j in range(4):
                n = JN[j]
                f = cw.tile([n, 64], F32R, tag=f"aw0f{j}", name=f"aw0f{j}")
                nc.vector.tensor_scalar(f[:], aw0b(j), s0[j][:], None, Alu.mult)
                aw0f.append(f)
                f = cw.tile([n, 48], F32R, tag=f"w1cf{j}", name=f"w1cf{j}")
                nc.vector.tensor_scalar(f[:], w1cb(j), s0[j][:], None, Alu.mult)
                w1cf.append(f)
            a0cps = sps.tile([64, 1], F32, tag="sp1", bufs=2)
            for j in range(4):
                nc.tensor.matmul(a0cps[:, :], aw0b(j), c0[j][:, :],
                                 start=(j == 0), stop=(j == 3))
            ab0p = cw.tile([64, 1], F32, tag="ab0p")
            nc.vector.tensor_add(ab0p[:], a0cps[:, :], vec(V_AB0)[0:64])
            b1ps = sps.tile([48, 1], F32, tag="sp1", bufs=2)
            for j in range(4):
                nc.tensor.matmul(b1ps[:, :], w1cb(j), c0[j][:, :],
                                 start=(j == 0), stop=(j == 3))
            b1c = cw.tile([48, 1], F32, tag="b1c")
            nc.vector.tensor_copy(b1c[:], b1ps[:, :])

            # ================= phase 2a: h1 (child contributions) ==============
            rhs0 = [tanh0[0], tanh0[1], tanh0[2], t3a]
            for bc in range(NB):
                bs = slice(BCW * bc, BCW * (bc + 1))
                h1ps = sps.tile([48, BCW], F32, tag="sp2", name=f"h1ps{bc}",
                                bufs=4)
                for j in range(4):
                    nc.tensor.matmul(h1ps[:, :], w1cf[j][:, :], rhs0[j][:JN[j], bs],
                                     start=(j == 0), stop=(j == 3))
                tm1 = tmp.tile([48, BCW], F32, tag="work", name=f"tm1{bc}", bufs=2)
                nc.vector.scalar_tensor_tensor(tm1[:, :], h1ps[:, :],
                                               b1c[:], rawh1[:, bs],
                                               Alu.add, Alu.add)
                nc.scalar.activation(tanh1[:, bs], tm1[:], Tanh)
                nc.vector.bn_stats(stats1[:, slice(6 * bc, 6 * bc + 6)],
                                   tanh1[:, bs])

            # ================= BN1 folds =================
            s1, c1 = bn_fold(stats1, 48, vec(V_G1), vec(V_BE1), "bn1")
            w2locf = cw.tile([48, 6], F32R, tag="w2locf")
            nc.vector.tensor_scalar(w2locf[:], w2locb, s1[:], None, Alu.mult)
            aw1f = cw.tile([48, 8], F32R, tag="aw1f")
            nc.vector.tensor_scalar(aw1f[:], aw1b, s1[:], None, Alu.mult)
            cps1 = sps.tile([8, 1], F32, tag="sp1", bufs=2)
            nc.tensor.matmul(cps1[:, :], aw1b, c1[:, :], start=True, stop=True)
            ab1p = cw.tile([8, 1], F32, tag="ab1p")
            nc.vector.tensor_add(ab1p[:], cps1[:, :], vec(V_AB1)[0:8])
            rps = sps.tile([6, 1], F32, tag="sp1", bufs=2)
            nc.tensor.matmul(rps[:, :], w2locb, c1[:, :], start=True, stop=True)
            rootc = cw.tile([6, 1], F32, tag="rootc")
            nc.vector.tensor_copy(rootc[:], rps[:, :])

            # ================= phase 3a: root partial + per-chunk bounce =======
            arin = drp.tile([6, B], F32)
            arout = drp.tile([6, B], F32)
            for bc in range(NB):
                bs = slice(BCW * bc, BCW * (bc + 1))
                rp = sps.tile([6, BCW], F32, tag="sp2", name=f"rps{bc}", bufs=4)
                rp_mm = nc.tensor.matmul(rp[:, :], w2locf[:, :], tanh1[:, bs],
                                         start=True, stop=True)
                nc.vector.scalar_tensor_tensor(h2p[:, bs], rp[:, :], rootc[:],
                                               rawroot[:, bs], Alu.add, Alu.add)

            # ================= phase 4: AllReduce of root partial =================
            nc.gpsimd.dma_start(arin[:], h2p[:])
            nc.gpsimd.collective_compute(
                "AllReduce", Alu.add,
                replica_groups=[list(range(N_CORES))],
                ins=[arin.opt()], outs=[arout.opt()])

            # ======== aux0 + aux1 heads, in flight during the AllReduce ========
            for bc in range(NB):
                bs = slice(BCW * bc, BCW * (bc + 1))
                a0ps = sps.tile([64, BCW], F32, tag="sp3", name=f"a0ps{bc}",
                                bufs=2)
                for j in range(4):
                    mmh = nc.tensor.matmul(a0ps[:, :], aw0f[j][:, :],
                                           rhs0[j][:JN[j], bs],
                                           start=(j == 0), stop=(j == 3))
                    if j == 0:
                        add_dep_helper(rp_mm.ins, mmh.ins, sync=False,
                                       reason="aux fills AllReduce flight")
                a0sb = tmp.tile([64, BCW], F32, tag="work", name=f"a0sb{bc}", bufs=2)
                nc.scalar.activation(a0sb[:], a0ps[:, :], Tanh, bias=ab0p[:])
                nc.vector.tensor_scalar(out_sb[0:64, bs], a0sb[:],
                                        vec(V_CW0)[0:64],
                                        vec(V_CB0)[0:64], Alu.mult, Alu.add)
                a1ps = sps.tile([8, BCW], F32, tag="sp3", name=f"a1ps{bc}", bufs=2)
                mmh1 = nc.tensor.matmul(a1ps[:, :], aw1f[:, :], tanh1[:, bs],
                                        start=True, stop=True)
                add_dep_helper(rp_mm.ins, mmh1.ins, sync=False,
                               reason="aux fills AllReduce flight")
                a1sb = tmp.tile([8, BCW], F32, tag="work", name=f"a1sb{bc}", bufs=2)
                nc.scalar.activation(a1sb[:], a1ps[:, :], Tanh, bias=ab1p[:])
                nc.vector.tensor_scalar(out_sb[64:72, bs], a1sb[:],
                                        vec(V_CW1)[0:8], vec(V_CB1)[0:8],
                                        Alu.mult, Alu.add)
            nc.gpsimd.dma_start(out_d[0:72, :], out_sb[0:72, :])
            h2s = big.tile([6, B], F32, tag="th0", name="h2s")
            nc.scalar.dma_start(h2s[:], arout[:])

            # ================= phase 5: root head =================
            tanh2 = big.tile([6, B], F32R, tag="th1", name="tanh2")
            nc.scalar.activation(tanh2[:], h2s[:], Tanh)
            for bc in range(NB):
                bs = slice(BCW * bc, BCW * (bc + 1))
                nc.vector.bn_stats(stats2[:, slice(6 * bc, 6 * bc + 6)],
                                   tanh2[:, bs])
            s2, c2 = bn_fold(stats2, 6, vec(V_G2), vec(V_BE2), "bn2")
            aw2ff = cw.tile([6, 2], F32R, tag="aw2ff")
            nc.vector.tensor_scalar(aw2ff[:], aw2f, s2[:], None, Alu.mult)
            hps = sps.tile([2, 1], F32, tag="sp1", bufs=2)
            nc.tensor.matmul(hps[:, :], aw2f, c2[:, :], start=True, stop=True)
            hdbp = cw.tile([2, 1], F32, tag="hdbp")
            nc.vector.tensor_add(hdbp[:], hps[:, :], vec(V_HDB)[0:2])
            for bc in range(NB):
                bs = slice(BCW * bc, BCW * (bc + 1))
                hp2 = sps.tile([2, BCW], F32, tag="sp2", name=f"hps{bc}", bufs=4)
                nc.tensor.matmul(hp2[:, :], aw2ff[:, :], tanh2[:, bs],
                                 start=True, stop=True)
                hsb = tmp.tile([2, BCW], F32, tag="work", name=f"hsb{bc}", bufs=2)
                nc.scalar.activation(hsb[:], hp2[:, :], Tanh, bias=hdbp[:])
                nc.vector.tensor_scalar(out_sb[96:98, bs], hsb[:],
                                        vec(V_HDS)[0:2], vec(V_HDSH)[0:2],
                                        Alu.mult, Alu.add)

            # ================= output (head rows) =================
            nc.gpsimd.dma_start(out_d[96:98, :], out_sb[96:98, :])

    nc.compile()
    return nc


def kernel(gene_input, params):
    global LAST_RESULTS
    from concourse import bass_utils

    if 'nc' not in _CACHE:
        _CACHE['nc'] = _build_program()
    nc = _CACHE['nc']

    per_core = _host_prep(gene_input, params)
    in_maps = [dict(xt=pc['xt'], wf=pc['wf'], sm=pc['sm']) for pc in per_core]

    res = bass_utils.run_bass_kernel_spmd(
        nc, in_maps, core_ids=list(range(N_CORES)),
        trace=bool(os.environ.get('KERNEL_TRACE')))
    LAST_RESULTS = res

    full = np.empty((B, T0 + T1 + 2), dtype=np.float32)
    for c in range(N_CORES):
        o = res.results[c]['out']
        full[:, LPC * c:LPC * (c + 1)] = o[0:64].T
        full[:, T0 + PPC * c:T0 + PPC * (c + 1)] = o[64:72].T
    o0 = res.results[0]['out']
    full[:, T0 + T1] = o0[96]
    full[:, T0 + T1 + 1] = o0[97]
    return full
